# revision 11
# baseline (speedup 1.0000x reference)
"""Trainium2 Bass kernel for ternary-weight linear (plinear STE forward).

Reference math:
    y = x @ ((w_pos > 0) - (w_neg > 0)).T      # [8192, 4096] fp32

The two binarized matmuls collapse into ONE matmul with a ternary {-1,0,1}
weight matrix. Ternary values are exact in bf16; x is quantized to bf16
(measured rel err ~1.2e-3, well under the 2e-2 gate). The ternary weight is
computed on host during input sharding -- the device kernel is a single
dense accumulated matmul.

Sharding (8 cores): TA=4 token shards x OB=2 out-feature shards; no
collectives, each core owns a disjoint [2048, 2048] output block.

Per-core schedule: x token-tiles of 256 are processed as two interleaved
128-token PSUM accumulation groups (8 single-bank groups of [128,512] fp32
live at once).  This paces PE consumption of W k-slices (1.7us/slice) just
above their DMA arrival rate (1.4us/slice) so the PE never starves while W
streams in, and staggers PSUM drains so group handoff is bubble-free.
W k-slices are separate tiles loaded on the sync queue; x tiles stream on
the scalar queue; output stores ride gpsimd; PSUM->SBUF drains on vector.
"""

import numpy as np
import ml_dtypes

P = 128
N_TOK, IN_F, OUT_F = 8192, 4096, 4096
K_SUB = IN_F // P             # 32 k-slices of 128
N_FREE = 512                  # one PSUM bank of fp32

TA, OB = 4, 2                 # token shards x out shards
T_TILE = 256                  # tokens per x tile (2 interleaved psum groups)
XC = 8                        # k-slices per x DMA chunk
W_FP8 = False                 # ship ternary W as fp8e4 (moving operand)
FP8DR = False                 # compensated fp8 DoubleRow mode

_CACHE = {}


def _build(repeats=1, ta=TA, ob=OB, t_tile=T_TILE, w_fp8=None, fp8dr=None):
    if w_fp8 is None:
        w_fp8 = W_FP8
    if fp8dr is None:
        fp8dr = FP8DR
    key = ("nc", repeats, ta, ob, t_tile, w_fp8, fp8dr)
    if key in _CACHE:
        return _CACHE[key]
    import concourse.bacc as bacc
    import concourse.mybir as mybir
    import concourse.tile as tile

    t_s = N_TOK // ta             # tokens per shard
    o_s = OUT_F // ob             # out features per shard
    n_tt = t_s // t_tile          # x tiles per shard
    m_sub = t_tile // P           # interleaved psum groups per x tile
    n_ob = o_s // N_FREE          # 512-wide out blocks
    assert m_sub * n_ob == 8      # exactly fill the 8 PSUM banks
    if fp8dr:
        # x shipped as (hi, lo) e4m3 planes; W duplicated across planes.
        # matmul computes sum_ki sum_i lhsT[ki,i,m]*rhs[ki,i,n]
        #   = sum_ki (x_hi+x_lo)[ki,m] * W[ki,n]  -- compensated precision.
        w_dt = x_dt = mybir.dt.float8e4
        pm = mybir.MatmulPerfMode.DoubleRow
    else:
        w_dt = mybir.dt.float8e4 if w_fp8 else mybir.dt.bfloat16
        x_dt = mybir.dt.bfloat16
        pm = None
    npl = 2 if fp8dr else 1       # planes

    nc = bacc.Bacc("TRN2", target_bir_lowering=False, debug=False)
    # pre-tiled on host: every DMA moves large per-partition-contiguous blocks
    xP = nc.dram_tensor("xP", (n_tt, P, K_SUB, npl, t_tile), x_dt,
                        kind="ExternalInput")
    wQ = nc.dram_tensor("wQ", (P, K_SUB, npl, o_s), w_dt,
                        kind="ExternalInput")
    y = nc.dram_tensor("y", (t_s, o_s), mybir.dt.float32, kind="ExternalOutput")

    y_r = y[:].rearrange("(to ti) o -> ti to o", ti=P)   # [128, t_s/128, o_s]

    with tile.TileContext(nc) as tc:
        with (
            tc.tile_pool(name="wst", bufs=K_SUB) as wst,
            tc.tile_pool(name="xp", bufs=6) as xp,
            tc.tile_pool(name="outp", bufs=(3 if o_s <= 2048 else 2)) as outp,
            tc.tile_pool(name="psum", bufs=8, space="PSUM") as psum_pool,
        ):
            for _rep in range(repeats):

                def load_x(tt):
                    # x tile as chunk-tiles of XC k-slices (scalar queue)
                    x_c = []
                    for ci in range(K_SUB // XC):
                        xt = xp.tile([P, XC, npl, t_tile], x_dt,
                                     tag="x", name=f"x{tt}_{ci}")
                        nc.scalar.dma_start(
                            xt[:], xP[tt, :, ci * XC:(ci + 1) * XC, :, :])
                        x_c.append(xt)
                    return x_c

                # x for the first two tiles ahead of the W stream
                x_pre = {tt: load_x(tt) for tt in range(min(2, n_tt))}
                # W: one tile per k-slice so matmul deps are per-slice no
                # matter how coarse region tracking is; even slices on the
                # sync queue, odd on scalar, so the stream lands ~2x faster.
                w_t = []
                for k in range(K_SUB):
                    wt = wst.tile([P, npl, o_s], w_dt, tag="w",
                                  name=f"w{k}")
                    eng = nc.sync if k % 2 == 0 else nc.scalar
                    eng.dma_start(wt[:], wQ[:, k, :, :])
                    w_t.append(wt)

                for tt in range(n_tt):
                    x_c = x_pre.pop(tt, None) or load_x(tt)
                    ps = [psum_pool.tile([P, N_FREE], mybir.dt.float32,
                                         tag="ps", name=f"ps{tt}_{g}")
                          for g in range(m_sub * n_ob)]
                    for k in range(K_SUB):
                        xs = x_c[k // XC]
                        for m in range(m_sub):
                            if fp8dr:
                                lhsT = xs[:, k % XC, :, m * P:(m + 1) * P]
                            else:
                                lhsT = xs[:, k % XC, 0, m * P:(m + 1) * P]
                            for j in range(n_ob):
                                if fp8dr:
                                    rhs = w_t[k][:, :,
                                                 j * N_FREE:(j + 1) * N_FREE]
                                else:
                                    rhs = w_t[k][:, 0,
                                                 j * N_FREE:(j + 1) * N_FREE]
                                nc.tensor.matmul(
                                    ps[m * n_ob + j][:],
                                    lhsT, rhs,
                                    start=(k == 0),
                                    stop=(k == K_SUB - 1),
                                    perf_mode=pm,
                                )
                    for m in range(m_sub):
                        o_t = outp.tile([P, o_s], mybir.dt.float32,
                                        tag="o", name=f"o{tt}_{m}")
                        for j in range(n_ob):
                            # drain each bank as its group stops, store the
                            # 512-block immediately -- keeps the tail short
                            nc.vector.tensor_copy(
                                o_t[:, j * N_FREE:(j + 1) * N_FREE],
                                ps[m * n_ob + j][:])
                            nc.gpsimd.dma_start(
                                y_r[:, tt * m_sub + m,
                                    j * N_FREE:(j + 1) * N_FREE],
                                o_t[:, j * N_FREE:(j + 1) * N_FREE])
    nc.compile()
    _CACHE[key] = nc
    return nc


def _shard_inputs(x, w_pos, w_neg, ta=TA, ob=OB, t_tile=T_TILE, w_fp8=None,
                  fp8dr=None):
    if w_fp8 is None:
        w_fp8 = W_FP8
    if fp8dr is None:
        fp8dr = FP8DR
    bf16 = ml_dtypes.bfloat16
    e4 = ml_dtypes.float8_e4m3
    w_np = e4 if (w_fp8 or fp8dr) else bf16
    t_s = N_TOK // ta
    o_s = OUT_F // ob
    n_tt = t_s // t_tile
    if fp8dr:
        # error-compensated split: x ~= hi + lo, both e4m3
        x_hi = x.astype(e4)
        x_lo = (x - x_hi.astype(np.float32)).astype(e4)
        xb = np.stack([x_hi, x_lo])                   # [2, N_TOK, IN_F]
        npl = 2
    else:
        xb = x.astype(bf16)[None]                     # [1, N_TOK, IN_F]
        npl = 1
    # ternary weight on host; exact in bf16/fp8
    w = ((w_pos > 0).astype(np.int8) - (w_neg > 0).astype(np.int8))
    wT = np.ascontiguousarray(w.T).astype(w_np)       # [IN_F, OUT_F]
    in_maps = []
    for c in range(8):
        tai, obi = divmod(c, ob)
        xs = xb[:, tai * t_s:(tai + 1) * t_s]         # [npl, t_s, IN_F]
        # [pl, (tt t), (ko ki)] -> [tt, ki, ko, pl, t]
        xp = np.ascontiguousarray(
            xs.reshape(npl, n_tt, t_tile, K_SUB, P)
            .transpose(1, 4, 3, 0, 2))
        # w: [in=(ko ki), o] -> [ki, ko, o] -> dup planes [ki, ko, npl, o]
        wq = (wT[:, obi * o_s:(obi + 1) * o_s]
              .reshape(K_SUB, P, o_s).transpose(1, 0, 2))
        wq = np.ascontiguousarray(
            np.broadcast_to(wq[:, :, None, :], (P, K_SUB, npl, o_s)))
        in_maps.append({"xP": xp, "wQ": wq})
    return in_maps


def _gather(results, ta=TA, ob=OB):
    t_s = N_TOK // ta
    o_s = OUT_F // ob
    y_full = np.empty((N_TOK, OUT_F), np.float32)
    for c in range(8):
        tai, obi = divmod(c, ob)
        y_full[tai * t_s:(tai + 1) * t_s,
               obi * o_s:(obi + 1) * o_s] = results[c]["y"]
    return y_full


def run(x, w_pos, w_neg, trace=False):
    """Returns (y_full, BassKernelResults)."""
    from concourse import bass_utils

    nc = _build()
    in_maps = _shard_inputs(x, w_pos, w_neg)
    res = bass_utils.run_bass_kernel_spmd(
        nc, in_maps, core_ids=list(range(8)), trace=trace
    )
    return _gather(res.results), res


def kernel(x, w_pos, w_neg):
    y, _ = run(x, w_pos, w_neg, trace=False)
    return y
